# revision 2
# baseline (speedup 1.0000x reference)
"""Causal self-attention (B=4, S=4096, D=64, H=4) on 8 TRN2 NeuronCores.

Sharding: the 16 (batch, head) pairs are distributed 2-per-core
(core c -> batch c//2, heads (2*(c%2), 2*(c%2)+1)). Each core runs the
full fused attention for its 2 pairs; no cross-core communication.

Per-core bass program (SPMD, identical shapes on all cores):
  - inputs: xT_aug [65, 4096] bf16 (x[b].T plus a ones row so the QKV
    biases come in through the matmul), wqk [65, 96] bf16 (per-pair
    [Wq_aug | 16 zero cols | Wk_aug] columns, q pre-scaled by
    1/sqrt(Dh); the zero gap puts k at a 32-aligned PSUM partition),
    wv [65, 32] bf16, mask [128, 128] f32 (strictly-lower-triangular
    -1e4 additive causal mask).
  - scores are computed TRANSPOSED (S.T = K_blk @ Q.T, key position on
    partitions) so the P@V contraction needs no transpose of P; the
    softmax denominator comes free from a 17th all-ones column in V.
    max-subtraction is skipped (scores are O(13), exp cannot overflow).
  - softmax exp is SPLIT ACROSS TWO ENGINES to break the ACT
    throughput wall (~123us for 18.9M exps at 1 elem/cycle/lane):
    diagonal (causal-masked) key blocks take exact ACT exp; off-
    diagonal blocks are load-balanced between ACT exact exp and a
    one-instruction DVE Schraudolph exp that emits bf16 BITS directly:
      bf16_bits(exp(s)) ~= int16(184.665*s + B2)
    (tensor_scalar mult+add, f32 PSUM in -> int16 SBUF out, the int16
    tile is bitcast back to bf16 for the P@V matmul). Keeping the
    diagonal blocks exact confines the ~3% Schraudolph weight error to
    wide softmax sums where it averages out (measured end-to-end rel
    err 0.009 vs 0.006 all-exact, tolerance 2e-2) and keeps -1e4
    masked scores away from the int16 conversion's saturation range.
  - scores per query super-block run phase A (all score matmuls,
    4-way row-tiled via tile_position with qT/kT replicated at
    partition offsets 0/32/64/96) then phase B (K=128 PV matmuls) so
    the PE stays continuously busy (HAM clock gate at 2.4 GHz).
  - output: [2 pairs, 17, 4096] f32 = unnormalized O.T rows 0..15 plus
    the softmax denominator in row 16; the division happens on host.
"""

import numpy as np
import ml_dtypes

_B, _S, _D = 4, 4096, 64
_H, _Dh = 4, 16
_NC = 8
_SCALE = 1.0 / np.sqrt(_Dh)
_MASK_NEG = -10000.0
_NQB = _S // 512  # 8 query super-blocks of 512
_NKB = _S // 128  # 32 key blocks of 128
_CHUNK = 3  # k-blocks per exp chunk (3 PSUM banks)

# Schraudolph exp -> bf16 bits: i16 = A2*s + B2, bitcast to bf16
_EXP_A2 = (2.0**23 / np.log(2.0)) / 65536.0
_EXP_B2 = (127.0 * 2.0**23 - 366393.0) / 65536.0

_cache = {}


def _build_nc():
    import concourse.tile as tile
    from concourse import bacc, mybir

    bf = mybir.dt.bfloat16
    i16 = mybir.dt.int16
    f32 = mybir.dt.float32
    Exp = mybir.ActivationFunctionType.Exp
    Mult = mybir.AluOpType.mult
    Add = mybir.AluOpType.add

    nc = bacc.Bacc("TRN2", target_bir_lowering=False, debug=False, num_devices=_NC)
    xT_d = nc.dram_tensor("xT", [_D + 1, _S], bf, kind="ExternalInput").ap()
    wqk_d = nc.dram_tensor("wqk", [_D + 1, 96], bf, kind="ExternalInput").ap()
    wv_d = nc.dram_tensor("wv", [_D + 1, 32], bf, kind="ExternalInput").ap()
    mask_d = nc.dram_tensor("mask", [128, 128], f32, kind="ExternalInput").ap()
    out_d = nc.dram_tensor("out", [2, 17, _S], f32, kind="ExternalOutput").ap()

    # greedy ACT/DVE balance: estimated busy ns accumulated per engine
    load = {"act": 0.0, "dve": 0.0}
    ACT_NS, DVE_NS = 1e9 / 1.2e9, 1e9 / 0.96e9  # per lane-element
    OVH = 170.0  # per-instruction overhead estimate

    def pick(act_cost, dve_cost):
        if load["act"] + act_cost <= load["dve"] + dve_cost:
            load["act"] += act_cost
            return "act"
        load["dve"] += dve_cost
        return "dve"

    with tile.TileContext(nc) as tc:
        with tc.tile_pool(name="singles", bufs=1) as singles:
            xT = singles.tile([_D + 1, _S], bf, tag="xT")
            wqk = singles.tile([_D + 1, 96], bf, tag="wqk")
            wv = singles.tile([_D + 1, 32], bf, tag="wv")
            maskt = singles.tile([128, 128], f32, tag="mask")
            # split across queues: each partition row is one DMA
            # descriptor, so a single transfer serializes ~65 of them
            for c in range(4):
                nc.sync.dma_start(
                    out=xT[:, 1024 * c : 1024 * (c + 1)],
                    in_=xT_d[:, 1024 * c : 1024 * (c + 1)],
                )
            nc.sync.dma_start(out=wqk[:], in_=wqk_d)
            nc.sync.dma_start(out=wv[:], in_=wv_d)
            nc.sync.dma_start(out=maskt[:], in_=mask_d)

            # qT/kT replicated at partition offsets 0/32/64/96 for 4-way
            # row-tiled score matmuls.
            qT = [singles.tile([128, _S], bf, tag=f"qT{p}", name=f"qT{p}") for p in range(2)]
            kT = [singles.tile([128, _S], bf, tag=f"kT{p}", name=f"kT{p}") for p in range(2)]
            V = [singles.tile([128, 17 * _NKB], bf, tag=f"V{p}", name=f"V{p}") for p in range(2)]
            for p in range(2):
                nc.vector.memset(V[p][:], 1.0)

            # ---- QKV projections ----
            # PSUM->SBUF copies split between DVE and ACT (ACT is otherwise
            # idle here); qT/kT replicated per chunk to partition offsets
            # 32/64/96 via small SBUF->SBUF DMAs so downstream deps release
            # incrementally.
            with tc.tile_pool(name="ps_proj", bufs=3, space="PSUM") as psA:
                for p in range(2):
                    for c in range(_S // 512):
                        csl = slice(512 * c, 512 * (c + 1))
                        pq = psA.tile([48, 512], f32, tag="qk")
                        nc.tensor.matmul(
                            pq[:],
                            wqk[:, 48 * p : 48 * p + 48],
                            xT[:, csl],
                            start=True,
                            stop=True,
                        )
                        nc.vector.tensor_copy(qT[p][0:16, csl], pq[0:16, :])
                        nc.scalar.copy(kT[p][0:16, csl], pq[32:48, :])
                        if c % 4 == 3:
                            # replicate finished half to offsets 32/64/96 on
                            # the idle gpsimd DMA queue
                            hsl = slice(2048 * (c // 4), 2048 * (c // 4 + 1))
                            for g in range(1, 4):
                                nc.gpsimd.dma_start(
                                    out=qT[p][32 * g : 32 * g + 16, hsl],
                                    in_=qT[p][0:16, hsl],
                                )
                                nc.gpsimd.dma_start(
                                    out=kT[p][32 * g : 32 * g + 16, hsl],
                                    in_=kT[p][0:16, hsl],
                                )
                for s in range(_NKB):
                    pv = psA.tile([128, 32], f32, tag="v")
                    nc.tensor.matmul(
                        pv[:],
                        xT[:, 128 * s : 128 * (s + 1)],
                        wv[:],
                        start=True,
                        stop=True,
                    )
                    nc.vector.tensor_copy(
                        V[0][:, 17 * s : 17 * s + 16], pv[:, 0:16]
                    )
                    nc.scalar.copy(V[1][:, 17 * s : 17 * s + 16], pv[:, 16:32])

            # ---- attention ----
            with (
                tc.tile_pool(name="ps_sc", bufs=2, space="PSUM") as ps_sc,
                tc.tile_pool(name="ps_o", bufs=2, space="PSUM") as ps_o,
                tc.tile_pool(name="ptp", bufs=2) as ptp,
                tc.tile_pool(name="stg", bufs=3) as stg,
            ):
                def emit_score_chunk(p, qi, pt, b0):
                    """One chunk of row-tiled score matmuls + mask + exp.

                    Off-diagonal block columns get either exact ACT exp or
                    DVE Schraudolph (greedy balance); diagonal blocks get
                    the DVE mask add + exact ACT exp.
                    """
                    nkb = 4 * qi + 4
                    qsl = slice(512 * qi, 512 * (qi + 1))
                    nblk = min(_CHUNK, nkb - b0)
                    ps = ps_sc.tile([128, 512 * _CHUNK], f32, tag="sc", name="ps")
                    ndiag = 0  # diag blocks in this chunk (always a suffix)
                    for t in range(nblk):
                        b = b0 + t
                        g = b % 4
                        nc.tensor.matmul(
                            ps[:, 512 * t : 512 * (t + 1)],
                            kT[p][32 * g : 32 * g + 16, 128 * b : 128 * (b + 1)],
                            qT[p][32 * g : 32 * g + 16, qsl],
                            start=True,
                            stop=True,
                            tile_position=(32 * g, 0),
                        )
                        j = b - 4 * qi
                        if j >= 0:  # diagonal block: causal mask
                            ndiag += 1
                            sl = ps[:, 512 * t + 128 * j : 512 * t + 128 * (j + 1)]
                            nc.vector.tensor_add(sl, sl, maskt[:])
                            load["dve"] += 128 * DVE_NS + OVH
                    noff = nblk - ndiag
                    if noff > 0:
                        eng = pick(512 * noff * ACT_NS + OVH, 512 * noff * DVE_NS + OVH)
                    else:
                        eng = None
                    if ndiag > 0:
                        load["act"] += 512 * ndiag * ACT_NS + OVH
                    if eng == "act":
                        # merge off-diag + diag into one exact ACT exp
                        nc.scalar.activation(
                            out=pt[:, 512 * b0 : 512 * (b0 + nblk)],
                            in_=ps[:, : 512 * nblk],
                            func=Exp,
                        )
                    else:
                        if eng == "dve":
                            pt_i16 = pt[:, 512 * b0 : 512 * (b0 + noff)].bitcast(i16)
                            nc.vector.tensor_scalar(
                                pt_i16,
                                ps[:, : 512 * noff],
                                _EXP_A2,
                                _EXP_B2,
                                Mult,
                                Add,
                            )
                        if ndiag > 0:
                            nc.scalar.activation(
                                out=pt[:, 512 * (b0 + noff) : 512 * (b0 + nblk)],
                                in_=ps[:, 512 * noff : 512 * nblk],
                                func=Exp,
                            )

                class BUnit:
                    """PV matmuls (K=128 keeps the PE clock warm) + store,
                    emitted incrementally so they interleave with the next
                    unit's score chunks."""

                    def __init__(self, p, qi, pt):
                        self.p, self.qi, self.pt = p, qi, pt
                        self.nkb = 4 * qi + 4
                        self.done = 0
                        self.po = ps_o.tile([17, 512], f32, tag="po", name="po")

                    def emit_upto(self, k):
                        for b in range(self.done, min(k, self.nkb)):
                            j = b - 4 * self.qi
                            qoff = 128 * j if j > 0 else 0
                            nc.tensor.matmul(
                                self.po[:, qoff:512],
                                V[self.p][:, 17 * b : 17 * b + 17],
                                self.pt[:, 512 * b + qoff : 512 * (b + 1)],
                                start=(b == 0),
                                stop=(b == self.nkb - 1),
                            )
                        self.done = max(self.done, min(k, self.nkb))

                    def finish(self):
                        self.emit_upto(self.nkb)
                        qsl = slice(512 * self.qi, 512 * (self.qi + 1))
                        ost = stg.tile([17, 512], f32, tag="ost", name="ost")
                        eng = pick(512 * ACT_NS + OVH, 512 * DVE_NS + OVH)
                        if eng == "act":
                            nc.scalar.copy(ost[:], self.po[:])
                        else:
                            nc.vector.tensor_copy(ost[:], self.po[:])
                        nc.sync.dma_start(out=out_d[self.p][:, qsl], in_=ost[:])

                # Fine-grained software pipeline: while emitting unit i's
                # score chunks (paced by the exp engines), interleave unit
                # i-1's PV matmuls proportionally so the PE never idles and
                # the PV density keeps the HAM clock gate warm.
                units = [(p, qi) for qi in reversed(range(_NQB)) for p in range(2)]
                prev = None
                for p, qi in units:
                    nkb = 4 * qi + 4
                    nchunks = (nkb + _CHUNK - 1) // _CHUNK
                    pt = ptp.tile([128, 512 * _NKB], bf, tag="pt", name="pt")
                    for c in range(nchunks):
                        # PV batch first: gives the PE ready work while the
                        # exp of the previous chunks drains the PSUM pool
                        if prev is not None:
                            prev.emit_upto(((c + 1) * prev.nkb) // nchunks)
                        emit_score_chunk(p, qi, pt, c * _CHUNK)
                    if prev is not None:
                        prev.finish()
                    prev = BUnit(p, qi, pt)
                prev.finish()

    nc.compile()
    return nc


def _get_nc():
    if "nc" not in _cache:
        _cache["nc"] = _build_nc()
    return _cache["nc"]


def _prepare_in_maps(x, Wq, bq, Wk, bk, Wv, bv):
    bf = ml_dtypes.bfloat16
    x = np.asarray(x, np.float32)
    ones = np.ones((1, _S), np.float32)

    def aug(W, b, h, scale=1.0):
        # [Dh, D+1] block for head h: weight rows plus bias column
        blk = np.concatenate(
            [W[h * _Dh : (h + 1) * _Dh, :], b[h * _Dh : (h + 1) * _Dh, None]], axis=1
        )
        return (blk * scale).T.astype(np.float32)  # [D+1, Dh]

    mask = np.where(
        np.arange(128)[:, None] > np.arange(128)[None, :], _MASK_NEG, 0.0
    ).astype(np.float32)

    in_maps = []
    for c in range(_NC):
        b_idx = c // 2
        heads = (2 * (c % 2), 2 * (c % 2) + 1)
        xT = np.concatenate([x[b_idx].T, ones], axis=0)  # [65, 4096]
        wqk_cols = []
        wv_cols = []
        zeros16 = np.zeros((_D + 1, _Dh), np.float32)
        for h in heads:
            wqk_cols.append(aug(Wq, bq, h, _SCALE))
            wqk_cols.append(zeros16)
            wqk_cols.append(aug(Wk, bk, h))
            wv_cols.append(aug(Wv, bv, h))
        in_maps.append(
            {
                "xT": xT.astype(bf),
                "wqk": np.concatenate(wqk_cols, axis=1).astype(bf),
                "wv": np.concatenate(wv_cols, axis=1).astype(bf),
                "mask": mask,
            }
        )
    return in_maps


def _assemble(results):
    final = np.empty((_B, _S, _D), np.float32)
    for c in range(_NC):
        b_idx = c // 2
        for p in range(2):
            h = 2 * (c % 2) + p
            o = np.asarray(results[c]["out"], np.float32)  # [2, 17, S]
            final[b_idx, :, h * _Dh : (h + 1) * _Dh] = (o[p, :16] / o[p, 16:17]).T
    return final


def _run(in_maps, trace=False, trace_kwargs=None):
    from concourse.bass_utils import run_bass_kernel_spmd

    nc = _get_nc()
    return run_bass_kernel_spmd(
        nc, in_maps, list(range(_NC)), trace=trace, **(trace_kwargs or {})
    )


def kernel(x, Wq, bq, Wk, bk, Wv, bv):
    in_maps = _prepare_in_maps(x, Wq, bq, Wk, bk, Wv, bv)
    res = _run(in_maps)
    return _assemble(res.results)


# revision 4
# speedup vs baseline: 1.1722x; 1.1722x over previous
"""Causal self-attention (B=4, S=4096, D=64, H=4) on 8 TRN2 NeuronCores.

Sharding: the 16 (batch, head) pairs are distributed 2-per-core
(core c -> batch c//2, heads (2*(c%2), 2*(c%2)+1)). Each core runs the
full fused attention for its 2 pairs; no cross-core communication.

Two tricks carry the kernel:

1. BILINEAR SCORES (K=64 keeps the PE's HAM clock gate warm). The
   TRN2 PE clock-gates to 1.2 GHz unless the activity monitor sees
   high-K matmuls (measured: K=16 4-way row-tiled matmuls never warm
   the clock, K>=64 does). Instead of Q@K^T with its K=16 (head dim)
   contraction, fold the projections into a host-precomputed bilinear
   form G_h = Wk_aug^T Wq_aug / sqrt(Dh) [65,65]:
       scores^T = x_aug G x_aug^T = x @ (G[0:64,:] @ x_aug^T)
   The per-query row G[64,:]@... cancels in softmax and is dropped.
   On device: Z_p = G64_p @ x_aug^T (K=65), then each score block is
   x^T_block @ Z_p with K=64, 2-way row-tiled (tile_position 0/64) --
   every matmul in the kernel now registers as HAM-busy, holding
   2.4 GHz, and the Q/K projection pass disappears entirely.

2. TWO-ENGINE SOFTMAX EXP (breaks the ACT throughput wall: 18.9M
   exps/core at 1 elem/cycle/lane is ~123us on ACT alone). Diagonal
   (causal-masked) key blocks take exact ACT exp; off-diagonal blocks
   are greedily balanced between ACT exact exp and a one-instruction
   DVE Schraudolph exp emitting bf16 BITS directly:
       bf16_bits(exp(s)) ~= int16(184.665*s + B2)
   (tensor_scalar mult+add, f32 PSUM in -> int16 SBUF out, bitcast
   back to bf16 for the P@V matmul). Keeping diagonal blocks exact
   confines the ~3% Schraudolph weight error to wide softmax sums
   where it averages out (measured rel err 0.011, tolerance 2e-2)
   and keeps -1e4 masked scores away from int16 saturation.

Scores are computed TRANSPOSED (key position on partitions) so P@V
needs no transpose; the softmax denominator comes free from a 17th
all-ones column in V; the final division happens on host. Exp skips
max-subtraction (scores are O(13), f32 exp cannot overflow).
"""

import numpy as np
import ml_dtypes

_B, _S, _D = 4, 4096, 64
_H, _Dh = 4, 16
_NC = 8
_SCALE = 1.0 / np.sqrt(_Dh)
_MASK_NEG = -10000.0
_NQB = _S // 512  # 8 query super-blocks of 512
_NKB = _S // 128  # 32 key blocks of 128
_CHUNK = 3  # k-blocks per exp chunk (3 PSUM banks)

# Schraudolph exp -> bf16 bits: i16 = A2*s + B2, bitcast to bf16
_EXP_A2 = (2.0**23 / np.log(2.0)) / 65536.0
_EXP_B2 = (127.0 * 2.0**23 - 366393.0) / 65536.0

_cache = {}


def _build_nc():
    import concourse.tile as tile
    from concourse import bacc, mybir

    bf = mybir.dt.bfloat16
    i16 = mybir.dt.int16
    f32 = mybir.dt.float32
    Exp = mybir.ActivationFunctionType.Exp
    Mult = mybir.AluOpType.mult
    Add = mybir.AluOpType.add

    nc = bacc.Bacc("TRN2", target_bir_lowering=False, debug=False, num_devices=_NC)
    xT_d = nc.dram_tensor("xT", [_D + 1, _S], bf, kind="ExternalInput").ap()
    xTc_d = nc.dram_tensor("xTc", [128, _S], bf, kind="ExternalInput").ap()
    gt_d = nc.dram_tensor("gt", [_D + 1, 128], bf, kind="ExternalInput").ap()
    wv_d = nc.dram_tensor("wv", [_D + 1, 32], bf, kind="ExternalInput").ap()
    mask_d = nc.dram_tensor("mask", [128, 128], f32, kind="ExternalInput").ap()
    out_d = nc.dram_tensor("out", [2, 17, _S], f32, kind="ExternalOutput").ap()

    # greedy ACT/DVE balance: estimated busy ns accumulated per engine
    load = {"act": 0.0, "dve": 0.0}
    ACT_NS, DVE_NS = 1e9 / 1.2e9, 1e9 / 0.96e9  # per lane-element
    OVH = 170.0  # per-instruction overhead estimate

    def pick(act_cost, dve_cost):
        if load["act"] + act_cost <= load["dve"] + dve_cost:
            load["act"] += act_cost
            return "act"
        load["dve"] += dve_cost
        return "dve"

    with tile.TileContext(nc) as tc:
        with tc.tile_pool(name="singles", bufs=1) as singles:
            xT = singles.tile([_D + 1, _S], bf, tag="xT")
            xTc = singles.tile([128, _S], bf, tag="xTc")
            gt = singles.tile([_D + 1, 128], bf, tag="gt")
            wv = singles.tile([_D + 1, 32], bf, tag="wv")
            maskt = singles.tile([128, 128], f32, tag="mask")
            # split across queues: each partition row is one DMA
            # descriptor, so a single transfer serializes ~65 of them
            for c in range(4):
                nc.sync.dma_start(
                    out=xT[:, 1024 * c : 1024 * (c + 1)],
                    in_=xT_d[:, 1024 * c : 1024 * (c + 1)],
                )
                nc.gpsimd.dma_start(
                    out=xTc[:, 1024 * c : 1024 * (c + 1)],
                    in_=xTc_d[:, 1024 * c : 1024 * (c + 1)],
                )
            nc.sync.dma_start(out=gt[:], in_=gt_d)
            nc.sync.dma_start(out=wv[:], in_=wv_d)
            nc.sync.dma_start(out=maskt[:], in_=mask_d)

            # Z_p replicated at partition offsets 0/64 for 2-way row-tiled
            # score matmuls (xTc comes replicated from host).
            Zt = [singles.tile([128, _S], bf, tag=f"Zt{p}", name=f"Zt{p}") for p in range(2)]
            V = [singles.tile([128, 17 * _NKB], bf, tag=f"V{p}", name=f"V{p}") for p in range(2)]
            for p in range(2):
                nc.vector.memset(V[p][:], 1.0)

            # ---- Z + V projections ----
            # PSUM->SBUF copies split between DVE and ACT; Z replicated to
            # partition offset 64 per finished half on the gpsimd DMA queue.
            with tc.tile_pool(name="ps_proj", bufs=3, space="PSUM") as psA:
                for p in range(2):
                    for c in range(_S // 512):
                        csl = slice(512 * c, 512 * (c + 1))
                        pz = psA.tile([_D, 512], f32, tag="z")
                        nc.tensor.matmul(
                            pz[:],
                            gt[:, 64 * p : 64 * p + 64],
                            xT[:, csl],
                            start=True,
                            stop=True,
                        )
                        if c % 2 == 0:
                            nc.vector.tensor_copy(Zt[p][0:64, csl], pz[:])
                        else:
                            nc.scalar.copy(Zt[p][0:64, csl], pz[:])
                        if c % 4 == 3:
                            hsl = slice(2048 * (c // 4), 2048 * (c // 4 + 1))
                            nc.gpsimd.dma_start(
                                out=Zt[p][64:128, hsl],
                                in_=Zt[p][0:64, hsl],
                            )
                for s in range(_NKB):
                    pv = psA.tile([128, 32], f32, tag="v")
                    nc.tensor.matmul(
                        pv[:],
                        xT[:, 128 * s : 128 * (s + 1)],
                        wv[:],
                        start=True,
                        stop=True,
                    )
                    nc.vector.tensor_copy(
                        V[0][:, 17 * s : 17 * s + 16], pv[:, 0:16]
                    )
                    nc.scalar.copy(V[1][:, 17 * s : 17 * s + 16], pv[:, 16:32])

            # ---- attention ----
            with (
                tc.tile_pool(name="ps_sc", bufs=2, space="PSUM") as ps_sc,
                tc.tile_pool(name="ps_o", bufs=2, space="PSUM") as ps_o,
                tc.tile_pool(name="ptp", bufs=2) as ptp,
                tc.tile_pool(name="stg", bufs=3) as stg,
            ):
                def emit_score_chunk(p, qi, pt, b0):
                    """One chunk of 2-way row-tiled K=64 score matmuls +
                    mask + engine-split exp."""
                    nkb = 4 * qi + 4
                    qsl = slice(512 * qi, 512 * (qi + 1))
                    nblk = min(_CHUNK, nkb - b0)
                    ps = ps_sc.tile([128, 512 * _CHUNK], f32, tag="sc", name="ps")
                    ndiag = 0  # diag blocks in this chunk (always a suffix)
                    for t in range(nblk):
                        b = b0 + t
                        g = 64 * (b % 2)
                        nc.tensor.matmul(
                            ps[:, 512 * t : 512 * (t + 1)],
                            xTc[g : g + 64, 128 * b : 128 * (b + 1)],
                            Zt[p][g : g + 64, qsl],
                            start=True,
                            stop=True,
                            tile_position=(g, 0),
                        )
                        j = b - 4 * qi
                        if j >= 0:  # diagonal block: causal mask
                            ndiag += 1
                            sl = ps[:, 512 * t + 128 * j : 512 * t + 128 * (j + 1)]
                            nc.vector.tensor_add(sl, sl, maskt[:])
                            load["dve"] += 128 * DVE_NS + OVH
                    noff = nblk - ndiag
                    if noff > 0:
                        eng = pick(512 * noff * ACT_NS + OVH, 512 * noff * DVE_NS + OVH)
                    else:
                        eng = None
                    if ndiag > 0:
                        load["act"] += 512 * ndiag * ACT_NS + OVH
                    if eng == "act":
                        # merge off-diag + diag into one exact ACT exp
                        nc.scalar.activation(
                            out=pt[:, 512 * b0 : 512 * (b0 + nblk)],
                            in_=ps[:, : 512 * nblk],
                            func=Exp,
                        )
                    else:
                        if eng == "dve":
                            pt_i16 = pt[:, 512 * b0 : 512 * (b0 + noff)].bitcast(i16)
                            nc.vector.tensor_scalar(
                                pt_i16,
                                ps[:, : 512 * noff],
                                _EXP_A2,
                                _EXP_B2,
                                Mult,
                                Add,
                            )
                        if ndiag > 0:
                            nc.scalar.activation(
                                out=pt[:, 512 * (b0 + noff) : 512 * (b0 + nblk)],
                                in_=ps[:, 512 * noff : 512 * nblk],
                                func=Exp,
                            )

                class BUnit:
                    """PV matmuls (K=128, HAM-busy) + store, emitted
                    incrementally so they interleave with the next unit's
                    score chunks."""

                    def __init__(self, p, qi, pt):
                        self.p, self.qi, self.pt = p, qi, pt
                        self.nkb = 4 * qi + 4
                        self.done = 0
                        self.po = ps_o.tile([17, 512], f32, tag="po", name="po")

                    def emit_upto(self, k):
                        for b in range(self.done, min(k, self.nkb)):
                            j = b - 4 * self.qi
                            qoff = 128 * j if j > 0 else 0
                            nc.tensor.matmul(
                                self.po[:, qoff:512],
                                V[self.p][:, 17 * b : 17 * b + 17],
                                self.pt[:, 512 * b + qoff : 512 * (b + 1)],
                                start=(b == 0),
                                stop=(b == self.nkb - 1),
                            )
                        self.done = max(self.done, min(k, self.nkb))

                    def finish(self):
                        self.emit_upto(self.nkb)
                        qsl = slice(512 * self.qi, 512 * (self.qi + 1))
                        ost = stg.tile([17, 512], f32, tag="ost", name="ost")
                        eng = pick(512 * ACT_NS + OVH, 512 * DVE_NS + OVH)
                        if eng == "act":
                            nc.scalar.copy(ost[:], self.po[:])
                        else:
                            nc.vector.tensor_copy(ost[:], self.po[:])
                        nc.sync.dma_start(out=out_d[self.p][:, qsl], in_=ost[:])

                # Fine-grained software pipeline: while emitting unit i's
                # score chunks (paced by the exp engines), interleave unit
                # i-1's PV matmuls proportionally so the PE never idles.
                units = [(p, qi) for qi in reversed(range(_NQB)) for p in range(2)]
                prev = None
                for p, qi in units:
                    nkb = 4 * qi + 4
                    nchunks = (nkb + _CHUNK - 1) // _CHUNK
                    pt = ptp.tile([128, 512 * _NKB], bf, tag="pt", name="pt")
                    for c in range(nchunks):
                        # PV batch first: gives the PE ready work while the
                        # exp of the previous chunks drains the PSUM pool
                        if prev is not None:
                            prev.emit_upto(((c + 1) * prev.nkb) // nchunks)
                        emit_score_chunk(p, qi, pt, c * _CHUNK)
                    if prev is not None:
                        prev.finish()
                    prev = BUnit(p, qi, pt)
                prev.finish()

    nc.compile()
    return nc


def _get_nc():
    if "nc" not in _cache:
        _cache["nc"] = _build_nc()
    return _cache["nc"]


def _prepare_in_maps(x, Wq, bq, Wk, bk, Wv, bv):
    bf = ml_dtypes.bfloat16
    x = np.asarray(x, np.float32)
    Wq, bq = np.asarray(Wq, np.float32), np.asarray(bq, np.float32)
    Wk, bk = np.asarray(Wk, np.float32), np.asarray(bk, np.float32)
    Wv, bv = np.asarray(Wv, np.float32), np.asarray(bv, np.float32)
    ones = np.ones((1, _S), np.float32)

    def aug(W, b, h, scale=1.0):
        # [Dh, D+1] block for head h: weight rows plus bias column
        blk = np.concatenate(
            [W[h * _Dh : (h + 1) * _Dh, :], b[h * _Dh : (h + 1) * _Dh, None]], axis=1
        )
        return (blk * scale).T.astype(np.float32)  # [D+1, Dh]

    mask = np.where(
        np.arange(128)[:, None] > np.arange(128)[None, :], _MASK_NEG, 0.0
    ).astype(np.float32)

    in_maps = []
    for c in range(_NC):
        b_idx = c // 2
        heads = (2 * (c % 2), 2 * (c % 2) + 1)
        xT = np.concatenate([x[b_idx].T, ones], axis=0)  # [65, 4096]
        xTc = np.concatenate([x[b_idx].T, x[b_idx].T], axis=0)  # [128, 4096]
        gt_cols = []
        wv_cols = []
        for h in heads:
            Wq_aug = aug(Wq, bq, h, _SCALE)  # [65, 16], q-scaled
            Wk_aug = aug(Wk, bk, h)  # [65, 16]
            G = Wk_aug @ Wq_aug.T  # [65, 65] = Wk_aug^T Wq_aug in row form
            gt_cols.append(G[0:64, :].T)  # G64^T = [65, 64]
            wv_cols.append(aug(Wv, bv, h))
        in_maps.append(
            {
                "xT": xT.astype(bf),
                "xTc": xTc.astype(bf),
                "gt": np.concatenate(gt_cols, axis=1).astype(bf),
                "wv": np.concatenate(wv_cols, axis=1).astype(bf),
                "mask": mask,
            }
        )
    return in_maps


def _assemble(results):
    final = np.empty((_B, _S, _D), np.float32)
    for c in range(_NC):
        b_idx = c // 2
        for p in range(2):
            h = 2 * (c % 2) + p
            o = np.asarray(results[c]["out"], np.float32)  # [2, 17, S]
            final[b_idx, :, h * _Dh : (h + 1) * _Dh] = (o[p, :16] / o[p, 16:17]).T
    return final


def _run(in_maps, trace=False, trace_kwargs=None):
    from concourse.bass_utils import run_bass_kernel_spmd

    nc = _get_nc()
    return run_bass_kernel_spmd(
        nc, in_maps, list(range(_NC)), trace=trace, **(trace_kwargs or {})
    )


def kernel(x, Wq, bq, Wk, bk, Wv, bv):
    in_maps = _prepare_in_maps(x, Wq, bq, Wk, bk, Wv, bv)
    res = _run(in_maps)
    return _assemble(res.results)


# revision 12
# speedup vs baseline: 1.1743x; 1.0018x over previous
"""Causal self-attention (B=4, S=4096, D=64, H=4) on 8 TRN2 NeuronCores.

Sharding: the 16 (batch, head) pairs are distributed 2-per-core
(core c -> batch c//2, heads (2*(c%2), 2*(c%2)+1)). Each core runs the
full fused attention for its 2 pairs; no cross-core communication.

Two tricks carry the kernel:

1. BILINEAR SCORES (K=64 keeps the PE's HAM clock gate warm). The
   TRN2 PE clock-gates to 1.2 GHz unless the activity monitor sees
   high-K matmuls (measured: K=16 4-way row-tiled matmuls never warm
   the clock, K>=64 does). Instead of Q@K^T with its K=16 (head dim)
   contraction, fold the projections into a host-precomputed bilinear
   form G_h = Wk_aug^T Wq_aug / sqrt(Dh) [65,65]:
       scores^T = x_aug G x_aug^T = x @ (G[0:64,:] @ x_aug^T)
   The per-query row G[64,:]@... cancels in softmax and is dropped.
   On device: Z_p = G64_p @ x_aug^T (K=65), then each score block is
   x^T_block @ Z_p with K=64, 2-way row-tiled (tile_position 0/64) --
   every matmul in the kernel now registers as HAM-busy, holding
   2.4 GHz, and the Q/K projection pass disappears entirely.

2. TWO-ENGINE SOFTMAX EXP (breaks the ACT throughput wall: 18.9M
   exps/core at 1 elem/cycle/lane is ~123us on ACT alone). Diagonal
   (causal-masked) key blocks take exact ACT exp; off-diagonal blocks
   are greedily balanced between ACT exact exp and a one-instruction
   DVE Schraudolph exp emitting bf16 BITS directly:
       bf16_bits(exp(s)) ~= int16(184.665*s + B2)
   (tensor_scalar mult+add, f32 PSUM in -> int16 SBUF out, bitcast
   back to bf16 for the P@V matmul). Keeping diagonal blocks exact
   confines the ~3% Schraudolph weight error to wide softmax sums
   where it averages out (measured rel err 0.011, tolerance 2e-2)
   and keeps -1e4 masked scores away from int16 saturation.

Scores are computed TRANSPOSED (key position on partitions) so P@V
needs no transpose; the softmax denominator comes free from a 17th
all-ones column in V; the final division happens on host. Exp skips
max-subtraction (scores are O(13), f32 exp cannot overflow).
"""

import numpy as np
import ml_dtypes

_B, _S, _D = 4, 4096, 64
_H, _Dh = 4, 16
_NC = 8
_SCALE = 1.0 / np.sqrt(_Dh)
_MASK_NEG = -10000.0
_NQB = _S // 512  # 8 query super-blocks of 512
_NKB = _S // 128  # 32 key blocks of 128
_CHUNK = 3  # k-blocks per exp chunk (3 PSUM banks)

# Schraudolph exp -> bf16 bits: i16 = A2*s + B2, bitcast to bf16
_EXP_A2 = (2.0**23 / np.log(2.0)) / 65536.0
_EXP_B2 = (127.0 * 2.0**23 - 366393.0) / 65536.0

_cache = {}


def _build_nc():
    import concourse.tile as tile
    from concourse import bacc, mybir

    bf = mybir.dt.bfloat16
    i16 = mybir.dt.int16
    f32 = mybir.dt.float32
    Exp = mybir.ActivationFunctionType.Exp
    Mult = mybir.AluOpType.mult
    Add = mybir.AluOpType.add

    nc = bacc.Bacc("TRN2", target_bir_lowering=False, debug=False, num_devices=_NC)
    xT_d = nc.dram_tensor("xT", [_D + 1, _S], bf, kind="ExternalInput").ap()
    gt_d = nc.dram_tensor("gt", [_D + 1, 128], bf, kind="ExternalInput").ap()
    wv_d = nc.dram_tensor("wv", [_D + 1, 32], bf, kind="ExternalInput").ap()
    mask_d = nc.dram_tensor("mask", [128, 128], f32, kind="ExternalInput").ap()
    out_d = nc.dram_tensor("out", [2, 17, _S], f32, kind="ExternalOutput").ap()

    # greedy ACT/DVE balance: estimated busy ns accumulated per engine
    load = {"act": 0.0, "dve": 0.0, "n": 0}
    ACT_NS, DVE_NS = 1e9 / 1.2e9, 1e9 / 0.96e9  # per lane-element
    OVH = 170.0  # per-instruction overhead estimate

    def pick(act_cost, dve_cost):
        if load["act"] + act_cost <= load["dve"] + dve_cost:
            load["act"] += act_cost
            return "act"
        load["dve"] += dve_cost
        return "dve"

    with tile.TileContext(nc) as tc:
        with tc.tile_pool(name="singles", bufs=1) as singles:
            xT = singles.tile([_D + 1, _S], bf, tag="xT")
            xTc = singles.tile([128, _S], bf, tag="xTc")
            gt = singles.tile([_D + 1, 128], bf, tag="gt")
            wv = singles.tile([_D + 1, 32], bf, tag="wv")
            maskt = singles.tile([128, 128], f32, tag="mask")
            # gt/wv first (tiny, unblock Z/V matmuls), xT split across the
            # sync and scalar HWDGE queues (each partition row is one DMA
            # descriptor, so a single transfer serializes ~65 of them);
            # odd-row score copy of x^T replicated SBUF->SBUF on the
            # gpsimd DGE as each chunk lands.
            nc.sync.dma_start(out=gt[:], in_=gt_d)
            nc.sync.dma_start(out=wv[:], in_=wv_d)
            for c in range(4):
                eng = nc.sync if c % 2 == 0 else nc.scalar
                eng.dma_start(
                    out=xT[:, 1024 * c : 1024 * (c + 1)],
                    in_=xT_d[:, 1024 * c : 1024 * (c + 1)],
                )
                nc.gpsimd.dma_start(
                    out=xTc[64:128, 1024 * c : 1024 * (c + 1)],
                    in_=xT[0:64, 1024 * c : 1024 * (c + 1)],
                )
            nc.scalar.dma_start(out=maskt[:], in_=mask_d)

            # Z_p replicated at partition offsets 0/64 for 2-way row-tiled
            # score matmuls (xTc comes replicated from host).
            Zt = [singles.tile([128, _S], bf, tag=f"Zt{p}", name=f"Zt{p}") for p in range(2)]
            V = [singles.tile([128, 17 * _NKB], bf, tag=f"V{p}", name=f"V{p}") for p in range(2)]
            for p in range(2):
                nc.vector.memset(V[p][:], 1.0)

            # ---- Z + V projections ----
            # PSUM->SBUF copies split between DVE and ACT; Z replicated to
            # partition offset 64 per finished half on the gpsimd DMA queue.
            with tc.tile_pool(name="ps_proj", bufs=3, space="PSUM") as psA:
                for p in range(2):
                    for c in range(_S // 512):
                        csl = slice(512 * c, 512 * (c + 1))
                        pz = psA.tile([_D, 512], f32, tag="z")
                        nc.tensor.matmul(
                            pz[:],
                            gt[:, 64 * p : 64 * p + 64],
                            xT[:, csl],
                            start=True,
                            stop=True,
                        )
                        if c % 2 == 0:
                            nc.vector.tensor_copy(Zt[p][0:64, csl], pz[:])
                        else:
                            nc.scalar.copy(Zt[p][0:64, csl], pz[:])
                        if c % 4 == 3:
                            hsl = slice(2048 * (c // 4), 2048 * (c // 4 + 1))
                            nc.gpsimd.dma_start(
                                out=Zt[p][64:128, hsl],
                                in_=Zt[p][0:64, hsl],
                            )
                for s in range(_NKB):
                    pv = psA.tile([128, 32], f32, tag="v")
                    nc.tensor.matmul(
                        pv[:],
                        xT[:, 128 * s : 128 * (s + 1)],
                        wv[:],
                        start=True,
                        stop=True,
                    )
                    nc.vector.tensor_copy(
                        V[0][:, 17 * s : 17 * s + 16], pv[:, 0:16]
                    )
                    nc.scalar.copy(V[1][:, 17 * s : 17 * s + 16], pv[:, 16:32])

            # ---- attention ----
            with (
                tc.tile_pool(name="ps_sc", bufs=2, space="PSUM") as ps_sc,
                tc.tile_pool(name="ps_o", bufs=2, space="PSUM") as ps_o,
                tc.tile_pool(name="ptp", bufs=2) as ptp,
                tc.tile_pool(name="stg", bufs=3) as stg,
            ):
                def emit_score_chunk(p, qi, pt, b0):
                    """One chunk of 2-way row-tiled K=64 score matmuls +
                    mask + engine-split exp."""
                    nkb = 4 * qi + 4
                    qsl = slice(512 * qi, 512 * (qi + 1))
                    nblk = min(_CHUNK, nkb - b0)
                    ps = ps_sc.tile([128, 512 * _CHUNK], f32, tag="sc", name="ps")
                    ndiag = 0  # diag blocks in this chunk (always a suffix)
                    for t in range(nblk):
                        b = b0 + t
                        g = 64 * (b % 2)
                        xsrc = xT if g == 0 else xTc
                        nc.tensor.matmul(
                            ps[:, 512 * t : 512 * (t + 1)],
                            xsrc[g : g + 64, 128 * b : 128 * (b + 1)],
                            Zt[p][g : g + 64, qsl],
                            start=True,
                            stop=True,
                            tile_position=(g, 0),
                        )
                        j = b - 4 * qi
                        if j >= 0:  # diagonal block: causal mask
                            ndiag += 1
                            sl = ps[:, 512 * t + 128 * j : 512 * t + 128 * (j + 1)]
                            nc.vector.tensor_add(sl, sl, maskt[:])
                            load["dve"] += 128 * DVE_NS + OVH
                    noff = nblk - ndiag
                    if noff > 0:
                        # strict alternation so consecutive chunks' exps
                        # overlap on the two engines
                        load["n"] += 1
                        eng = "dve" if load["n"] % 2 == 0 else "act"
                        load[eng] += 512 * noff * (DVE_NS if eng == "dve" else ACT_NS) + OVH
                    else:
                        eng = None
                    if ndiag > 0:
                        load["act"] += 512 * ndiag * ACT_NS + OVH
                    if eng == "act":
                        # merge off-diag + diag into one exact ACT exp
                        nc.scalar.activation(
                            out=pt[:, 512 * b0 : 512 * (b0 + nblk)],
                            in_=ps[:, : 512 * nblk],
                            func=Exp,
                        )
                    else:
                        if eng == "dve":
                            pt_i16 = pt[:, 512 * b0 : 512 * (b0 + noff)].bitcast(i16)
                            nc.vector.tensor_scalar(
                                pt_i16,
                                ps[:, : 512 * noff],
                                _EXP_A2,
                                _EXP_B2,
                                Mult,
                                Add,
                            )
                        if ndiag > 0:
                            nc.scalar.activation(
                                out=pt[:, 512 * (b0 + noff) : 512 * (b0 + nblk)],
                                in_=ps[:, 512 * noff : 512 * nblk],
                                func=Exp,
                            )

                class BUnit:
                    """PV matmuls (K=128, HAM-busy) + store, emitted
                    incrementally so they interleave with the next unit's
                    score chunks."""

                    def __init__(self, p, qi, pt):
                        self.p, self.qi, self.pt = p, qi, pt
                        self.nkb = 4 * qi + 4
                        self.done = 0
                        self.po = ps_o.tile([17, 512], f32, tag="po", name="po")

                    def emit_upto(self, k):
                        for b in range(self.done, min(k, self.nkb)):
                            j = b - 4 * self.qi
                            qoff = 128 * j if j > 0 else 0
                            nc.tensor.matmul(
                                self.po[:, qoff:512],
                                V[self.p][:, 17 * b : 17 * b + 17],
                                self.pt[:, 512 * b + qoff : 512 * (b + 1)],
                                start=(b == 0),
                                stop=(b == self.nkb - 1),
                            )
                        self.done = max(self.done, min(k, self.nkb))

                    def finish(self):
                        self.emit_upto(self.nkb)
                        qsl = slice(512 * self.qi, 512 * (self.qi + 1))
                        ost = stg.tile([17, 512], f32, tag="ost", name="ost")
                        eng = pick(512 * ACT_NS + OVH, 512 * DVE_NS + OVH)
                        if eng == "act":
                            nc.scalar.copy(ost[:], self.po[:])
                        else:
                            nc.vector.tensor_copy(ost[:], self.po[:])
                        nc.sync.dma_start(out=out_d[self.p][:, qsl], in_=ost[:])

                # Fine-grained software pipeline: while emitting unit i's
                # score chunks (paced by the exp engines), interleave unit
                # i-1's PV matmuls proportionally so the PE never idles.
                units = [(p, qi) for qi in reversed(range(_NQB)) for p in range(2)]
                prev = None
                for p, qi in units:
                    nkb = 4 * qi + 4
                    nchunks = (nkb + _CHUNK - 1) // _CHUNK
                    pt = ptp.tile([128, 512 * _NKB], bf, tag="pt", name="pt")
                    unit = BUnit(p, qi, pt)
                    for c in range(nchunks):
                        # PV batch first: gives the PE ready work while the
                        # exp of the previous chunks drains the PSUM pool
                        if prev is not None:
                            prev.emit_upto(((c + 1) * prev.nkb) // nchunks)
                        emit_score_chunk(p, qi, pt, c * _CHUNK)
                        if prev is None and c >= 1:
                            # first unit: self-paced PV (1-chunk lag) so the
                            # PE has K=128 work from the start (HAM warm-up)
                            unit.emit_upto(c * _CHUNK)
                    if prev is not None:
                        prev.finish()
                    prev = unit
                prev.finish()

    nc.compile()
    return nc


def _get_nc():
    if "nc" not in _cache:
        _cache["nc"] = _build_nc()
    return _cache["nc"]


def _prepare_in_maps(x, Wq, bq, Wk, bk, Wv, bv):
    bf = ml_dtypes.bfloat16
    x = np.asarray(x, np.float32)
    Wq, bq = np.asarray(Wq, np.float32), np.asarray(bq, np.float32)
    Wk, bk = np.asarray(Wk, np.float32), np.asarray(bk, np.float32)
    Wv, bv = np.asarray(Wv, np.float32), np.asarray(bv, np.float32)
    ones = np.ones((1, _S), np.float32)

    def aug(W, b, h, scale=1.0):
        # [Dh, D+1] block for head h: weight rows plus bias column
        blk = np.concatenate(
            [W[h * _Dh : (h + 1) * _Dh, :], b[h * _Dh : (h + 1) * _Dh, None]], axis=1
        )
        return (blk * scale).T.astype(np.float32)  # [D+1, Dh]

    mask = np.where(
        np.arange(128)[:, None] > np.arange(128)[None, :], _MASK_NEG, 0.0
    ).astype(np.float32)

    in_maps = []
    for c in range(_NC):
        b_idx = c // 2
        heads = (2 * (c % 2), 2 * (c % 2) + 1)
        xT = np.concatenate([x[b_idx].T, ones], axis=0)  # [65, 4096]
        gt_cols = []
        wv_cols = []
        for h in heads:
            Wq_aug = aug(Wq, bq, h, _SCALE)  # [65, 16], q-scaled
            Wk_aug = aug(Wk, bk, h)  # [65, 16]
            G = Wk_aug @ Wq_aug.T  # [65, 65] = Wk_aug^T Wq_aug in row form
            gt_cols.append(G[0:64, :].T)  # G64^T = [65, 64]
            wv_cols.append(aug(Wv, bv, h))
        in_maps.append(
            {
                "xT": xT.astype(bf),
                "gt": np.concatenate(gt_cols, axis=1).astype(bf),
                "wv": np.concatenate(wv_cols, axis=1).astype(bf),
                "mask": mask,
            }
        )
    return in_maps


def _assemble(results):
    final = np.empty((_B, _S, _D), np.float32)
    for c in range(_NC):
        b_idx = c // 2
        for p in range(2):
            h = 2 * (c % 2) + p
            o = np.asarray(results[c]["out"], np.float32)  # [2, 17, S]
            final[b_idx, :, h * _Dh : (h + 1) * _Dh] = (o[p, :16] / o[p, 16:17]).T
    return final


def _run(in_maps, trace=False, trace_kwargs=None):
    from concourse.bass_utils import run_bass_kernel_spmd

    nc = _get_nc()
    return run_bass_kernel_spmd(
        nc, in_maps, list(range(_NC)), trace=trace, **(trace_kwargs or {})
    )


def kernel(x, Wq, bq, Wk, bk, Wv, bv):
    in_maps = _prepare_in_maps(x, Wq, bq, Wk, bk, Wv, bv)
    res = _run(in_maps)
    return _assemble(res.results)
